# revision 20
# baseline (speedup 1.0000x reference)
"""Trainium2 Bass kernel for the DAGH sample loss.

loss = 0.5 * tr_loss / n^2 * 1e4 + 0.5 * bla_loss / n + 0.5 * oth_loss / K

with
  tr_loss  = dot(rowsum(w), fn) + dot(colsum(w), bn) - 2 * sum((F @ w) * B)
  oth_loss = ||F F^T / n - I||_F^2
  bla_loss = sum_k (sum_i F[k, i])^2

Strategy (8 cores, SPMD): shard w row-wise (1024 rows/core) and F
column-wise to match.  All three w-dependent reductions collapse into a
single augmented matmul per core:

  Faug = [F_loc; ones; fn_loc]  (66 x 1024)   ->   out = Faug @ w_shard

  rows 0..63 . B      -> partial cross
  row  64    . bn     -> partial colsum-dot
  row  65    . ones   -> partial rowsum-dot

The elementwise-multiply-and-reduce against Baug = [B; bn; 1] runs on the
vector engine (fused tensor_tensor_reduce) while the tensor engine streams
the next w tile.  Gram (F_loc F_loc^T) and row-sum partials for
oth/bla_loss come from a few extra tiny matmuls on the already-loaded
transposed F tiles.  Host combines the 8 cores' scalar partials in f64.
"""

import numpy as np

BATCH = 8192
K = 64
NCORES = 8
ROWS = BATCH // NCORES  # w rows per core
KP = 128  # matmul contraction tile (partition dim)
NI = ROWS // KP  # i-tiles per core (8)
JT = 512  # j-tile (psum free dim)
NJ = BATCH // JT  # j-tiles (16)
M = K + 2  # augmented lhs rows (F, ones, fn)

MM_DTYPE = "float32r"  # tensor-engine dtype: "float32" (exact) or "float32r" (fast)

_compiled = {}


def _build(loop_reps=1, runtime_reps=False, dma_only=False):
    """loop_reps > 1 (or runtime_reps=True with a "reps" input tensor)
    wraps the main stream in a hardware For_i loop that recomputes
    identical results -- used only by test.py to time the steady-state
    stream without per-call NEFF-load overhead.  dma_only strips compute
    to measure the pure DMA bandwidth ceiling."""
    import contextlib

    import concourse.bacc as bacc
    import concourse.mybir as mybir
    import concourse.tile as tile

    mm_dt = getattr(mybir.dt, MM_DTYPE)
    f32 = mybir.dt.float32

    nc = bacc.Bacc(
        "TRN2", target_bir_lowering=False, debug=False, num_devices=NCORES
    )
    bf16 = mybir.dt.bfloat16

    w_d = nc.dram_tensor("w", [ROWS, BATCH], mm_dt, kind="ExternalInput").ap()
    ft_d = nc.dram_tensor("ft", [ROWS, M], mm_dt, kind="ExternalInput").ap()
    # B block in bf16 (cross term is insensitive); bn + ones rows in f32
    bb_d = nc.dram_tensor("bb", [K, BATCH], bf16, kind="ExternalInput").ap()
    bno_d = nc.dram_tensor("bno", [2, BATCH], f32, kind="ExternalInput").ap()
    ones_d = nc.dram_tensor("ones", [KP, 2], mm_dt, kind="ExternalInput").ap()
    reps_d = None
    if runtime_reps:
        reps_d = nc.dram_tensor(
            "reps", [1, 2], mybir.dt.int32, kind="ExternalInput"
        ).ap()
    acc_d = nc.dram_tensor("acc", [M, NJ], f32, kind="ExternalOutput").ap()
    gram_d = nc.dram_tensor("gram", [K, K], f32, kind="ExternalOutput").ap()
    rs_d = nc.dram_tensor("rs", [M, 2], f32, kind="ExternalOutput").ap()

    w_r = w_d.rearrange("(a p) n -> a p n", p=KP)
    ft_r = ft_d.rearrange("(a p) m -> a p m", p=KP)

    with tile.TileContext(nc) as tc:
        with (
            tc.tile_pool(name="persist", bufs=1) as persist,
            tc.tile_pool(name="ftp", bufs=NI) as ftp,
            tc.tile_pool(name="wp", bufs=12) as wp,
            tc.tile_pool(name="scratch", bufs=3) as scratch,
            tc.tile_pool(name="psum", bufs=4, space="PSUM") as psum,
            tc.tile_pool(name="psum_small", bufs=2, space="PSUM") as psum_small,
        ):
            bb_sb = persist.tile([K, BATCH], bf16, name="bb_sb")
            nc.sync.dma_start(out=bb_sb, in_=bb_d)
            bno_sb = persist.tile([2, BATCH], f32, name="bno_sb")
            nc.sync.dma_start(out=bno_sb, in_=bno_d)

            ft_tiles = []
            for i in range(NI):
                t = ftp.tile([KP, M], mm_dt, name=f"ft{i}")
                nc.sync.dma_start(out=t, in_=ft_r[i])
                ft_tiles.append(t)

            ones_sb = persist.tile([KP, 2], mm_dt, name="ones_sb")
            nc.sync.dma_start(out=ones_sb, in_=ones_d)

            acc_sb = persist.tile([M, NJ], f32, name="acc_sb")
            if dma_only:
                nc.vector.memset(acc_sb, 0.0)

            # main stream: for each j-tile accumulate Faug @ w over i-tiles,
            # then multiply-by-Baug + reduce on the vector engine.
            if runtime_reps:
                reps_sb = persist.tile([1, 2], mybir.dt.int32, name="reps_sb")
                nc.sync.dma_start(out=reps_sb, in_=reps_d)
                nreps = nc.values_load(
                    reps_sb[0:1, 0:1], min_val=0, max_val=1 << 20
                )
                rep_ctx = tc.For_i(0, nreps, 1)
            elif loop_reps > 1:
                rep_ctx = tc.For_i(0, loop_reps, 1)
            else:
                rep_ctx = contextlib.nullcontext()
            with rep_ctx:
                for j in range(NJ):
                    if dma_only:
                        for i in range(NI):
                            wt = wp.tile([KP, JT], mm_dt, name="wtile")
                            nc.sync.dma_start(
                                out=wt, in_=w_r[i, :, j * JT : (j + 1) * JT]
                            )
                        continue
                    pt = psum.tile([M, JT], f32, name="mm_out")
                    for i in range(NI):
                        wt = wp.tile([KP, JT], mm_dt, name="wtile")
                        nc.sync.dma_start(
                            out=wt, in_=w_r[i, :, j * JT : (j + 1) * JT]
                        )
                        nc.tensor.matmul(
                            pt,
                            lhsT=ft_tiles[i],
                            rhs=wt,
                            start=(i == 0),
                            stop=(i == NI - 1),
                        )
                    # note: fused tensor_tensor_reduce faults on HW with a
                    # PSUM input, so multiply and reduce as separate DVE ops
                    st = scratch.tile([M, JT], f32, name="ttr_out")
                    nc.vector.tensor_mul(
                        st[0:K], pt[0:K], bb_sb[:, j * JT : (j + 1) * JT]
                    )
                    nc.vector.tensor_mul(
                        st[K : K + 2],
                        pt[K : K + 2],
                        bno_sb[:, j * JT : (j + 1) * JT],
                    )
                    nc.vector.tensor_reduce(
                        out=acc_sb[:, j : j + 1],
                        in_=st,
                        axis=mybir.AxisListType.X,
                        op=mybir.AluOpType.add,
                    )
            nc.sync.dma_start(out=acc_d, in_=acc_sb)

            # gram partial: F_loc F_loc^T accumulated over i-tiles
            gram_pt = psum_small.tile([K, K], f32, name="gram_pt")
            for i in range(NI):
                nc.tensor.matmul(
                    gram_pt,
                    lhsT=ft_tiles[i][:, 0:K],
                    rhs=ft_tiles[i][:, 0:K],
                    start=(i == 0),
                    stop=(i == NI - 1),
                )
            gram_sb = persist.tile([K, K], f32, name="gram_sb")
            nc.vector.tensor_copy(gram_sb, gram_pt)
            nc.sync.dma_start(out=gram_d, in_=gram_sb)

            # row-sum partial of Faug (rows 0..63 give rs for bla_loss);
            # N=2 (duplicated ones column) — fp32r needs an even free size
            rs_pt = psum_small.tile([M, 2], f32, name="rs_pt")
            for i in range(NI):
                nc.tensor.matmul(
                    rs_pt,
                    lhsT=ft_tiles[i],
                    rhs=ones_sb,
                    start=(i == 0),
                    stop=(i == NI - 1),
                )
            rs_sb = persist.tile([M, 2], f32, name="rs_sb")
            nc.vector.tensor_copy(rs_sb, rs_pt)
            nc.sync.dma_start(out=rs_d, in_=rs_sb)

    nc.compile()
    return nc


def _get_program():
    if "nc" not in _compiled:
        _compiled["nc"] = _build()
    return _compiled["nc"]


def _make_in_maps(w_batch, F_batch, B_batch):
    w_batch = np.asarray(w_batch, dtype=np.float32)
    F_batch = np.asarray(F_batch, dtype=np.float32)
    B_batch = np.asarray(B_batch, dtype=np.float32)

    from concourse import mybir

    np_bf16 = mybir.dt.np(mybir.dt.bfloat16)

    fn = (F_batch.astype(np.float64) ** 2).sum(axis=0)  # [n] col sq-norms of F
    bn = (B_batch.astype(np.float64) ** 2).sum(axis=0)  # [n] col sq-norms of B

    bb = B_batch.astype(np_bf16)
    bno = np.empty((2, BATCH), dtype=np.float32)
    bno[0] = bn.astype(np.float32)
    bno[1] = 1.0

    ones = np.ones((KP, 2), dtype=np.float32)

    in_maps = []
    for c in range(NCORES):
        lo, hi = c * ROWS, (c + 1) * ROWS
        ft = np.empty((ROWS, M), dtype=np.float32)
        ft[:, 0:K] = F_batch[:, lo:hi].T
        ft[:, K] = 1.0
        ft[:, K + 1] = fn[lo:hi].astype(np.float32)
        in_maps.append(
            {"w": w_batch[lo:hi], "ft": ft, "bb": bb, "bno": bno, "ones": ones}
        )
    return in_maps


def _combine(results):
    n = float(BATCH)
    S = np.zeros(M, dtype=np.float64)
    gram = np.zeros((K, K), dtype=np.float64)
    rs = np.zeros(K, dtype=np.float64)
    for r in results:
        S += r["acc"].astype(np.float64).sum(axis=1)
        gram += r["gram"].astype(np.float64)
        rs += r["rs"][0:K, 0].astype(np.float64)

    cross = S[0:K].sum()
    colsum_dot = S[K]
    rowsum_dot = S[K + 1]
    tr_loss = rowsum_dot + colsum_dot - 2.0 * cross

    g = gram / n - np.eye(K, dtype=np.float64)
    oth_loss = (g * g).sum()
    bla_loss = (rs * rs).sum()

    loss = (
        0.5 * tr_loss / (n * n) * 10000.0
        + 0.5 * bla_loss / n
        + 0.5 * oth_loss / K
    )
    return np.float32(loss)


def _ping_devices():
    """Touch every core with a trivial op first: a device wedged by a
    previously crashed process fails its next operation once and then
    recovers, so absorb that failure here instead of in the real run."""
    import time

    import jax

    for _ in range(3):
        try:
            for d in jax.devices()[:NCORES]:
                x = jax.device_put(np.ones(4, np.float32), d)
                (x + 1.0).block_until_ready()
            return
        except Exception:
            time.sleep(2.0)


def kernel(w_batch, F_batch, B_batch):
    import time

    from concourse.bass_utils import run_bass_kernel_spmd

    nc = _get_program()
    in_maps = _make_in_maps(w_batch, F_batch, B_batch)
    _ping_devices()
    try:
        res = run_bass_kernel_spmd(nc, in_maps, core_ids=list(range(NCORES)))
    except Exception:
        time.sleep(2.0)
        _ping_devices()
        res = run_bass_kernel_spmd(nc, in_maps, core_ids=list(range(NCORES)))
    return _combine(res.results)
